# revision 31
# baseline (speedup 1.0000x reference)
"""MiniGPT forward pass on 8 Trainium2 NeuronCores (Bass/Tile).

Sharding: token-sharded transformer (core r owns batch-0 chunk r and batch-1
chunk 7-r, 256 tokens each), per-layer KV AllGather, attention over the full
gathered key set with per-core causal chunk bias folded into the exp bias,
and the on-diagonal 256x256 block computed from local K/V with a constant
triangular mask.  lm_head is vocab-sharded (tied wte) after a final
AllGather of the normalized activations.  Matmuls run in bf16 with fp32
PSUM accumulation; logits are written in bf16 and upcast on the host.

Device layout: the residual stream is channel-major ("transposed"):
xT[p, sub, t] with channel c = sub*128 + p; every matmul consumes/produces
channel-major tensors so the residual never needs an on-device transpose.
"""

import sys

sys.path.insert(0, "/opt/trn_rl_repo")

from contextlib import ExitStack

import numpy as np
from ml_dtypes import bfloat16

import concourse.bass as bass  # noqa: F401
import concourse.tile as tile
from concourse import bacc, mybir
from concourse.bass_utils import run_bass_kernel_spmd
from concourse.masks import make_identity

V, BLK, L, H, C, FF = 50257, 2048, 6, 8, 512, 2048
D = C // H  # 64
B, T = 2, 2048
R = 8
TLOC = 512  # tokens per core
CH = 256  # chunk size
VC = 6283  # vocab slice per core (8*6283 >= 50257)
NEG = -30.0

F32 = mybir.dt.float32
BF16 = mybir.dt.bfloat16


def _to_cpart(a):
    """[C, N] -> [128, C//128, N] with c = sub*128 + p."""
    Cdim, N = a.shape
    return np.ascontiguousarray(a.reshape(Cdim // 128, 128, N).transpose(1, 0, 2))


def _chunks_for_core(r):
    return r, 7 - r


# ---------------------------------------------------------------------------
# Device program
# ---------------------------------------------------------------------------

def build_program():
    nc = bacc.Bacc("TRN2", target_bir_lowering=False, debug=False,
                   num_devices=R)

    def din(name, shape, dt):
        return nc.dram_tensor(name, shape, dt, kind="ExternalInput").ap()

    io = {
        "x0T": din("x0T", [128, 4, TLOC], F32),
        "wqkvT": din("wqkvT", [L, 128, 4, 3 * C], BF16),
        "wcT": din("wcT", [L, 64, H, C], BF16),
        "wfcT": din("wfcT", [L, 128, 4, FF], BF16),
        "wprT": din("wprT", [L, 128, 16, C], BF16),
        "qkv_b": din("qkv_b", [L, 128, 9], F32),
        "fc_b": din("fc_b", [L, 128, 16], F32),
        "attn_bias": din("attn_bias", [128, 2, 16], F32),
        "trimask": din("trimask", [128, 2, CH], BF16),
        "wteT": din("wteT", [128, 4, VC], BF16),
        "logits": nc.dram_tensor("logits", [4096, VC], BF16,
                                 kind="ExternalOutput").ap(),
        "rg": [list(range(R))],
    }

    with tile.TileContext(nc) as tc:
        _build_body(nc, tc, io)
    nc.compile()
    return nc


def _build_body(nc, tc, io):
    x0T, wqkvT, wcT, wfcT, wprT = (io["x0T"], io["wqkvT"], io["wcT"],
                                   io["wfcT"], io["wprT"])
    qkv_b, fc_b = io["qkv_b"], io["fc_b"]
    attn_bias, trimask = io["attn_bias"], io["trimask"]
    wteT, logits, rg = io["wteT"], io["logits"], io["rg"]

    ctx = ExitStack()
    const = ctx.enter_context(tc.tile_pool(name="const", bufs=1))
    dram = ctx.enter_context(tc.tile_pool(name="dram", bufs=1, space="DRAM"))

    xT = const.tile([128, 4, TLOC], F32)
    nc.sync.dma_start(xT[:], x0T[:])

    ones4 = const.tile([128, 4, 1], BF16)
    nc.vector.memset(ones4[:], 1.0)
    bias_sb = const.tile([128, 2, 16], F32)
    nc.sync.dma_start(bias_sb[:], attn_bias[:])
    tri_sb = const.tile([128, 2, CH], BF16)
    nc.sync.dma_start(tri_sb[:], trimask[:])
    qkvb_sb = const.tile([128, L, 9], F32)
    nc.sync.dma_start(qkvb_sb[:], qkv_b.rearrange("l p n -> p l n"))
    fcb_sb = const.tile([128, L, 16], F32)
    nc.sync.dma_start(fcb_sb[:], fc_b.rearrange("l p n -> p l n"))
    eps_sb = const.tile([1, 1], F32)
    nc.vector.memset(eps_sb[:], 1e-5)
    zero_sb = const.tile([128, 1], F32)
    nc.vector.memset(zero_sb[:], 0.0)

    layer_ctx = ExitStack()
    lp = layer_ctx.enter_context
    wpool = lp(tc.tile_pool(name="wpool", bufs=1))
    acts = lp(tc.tile_pool(name="acts", bufs=2))   # xh tiles
    acts1 = lp(tc.tile_pool(name="acts1", bufs=1))  # qT/kT/vaug/yT/hT
    kvp = lp(tc.tile_pool(name="kvp", bufs=1))
    expp = lp(tc.tile_pool(name="expp", bufs=4))
    stat = lp(tc.tile_pool(name="stat", bufs=2))
    ps_sc = lp(tc.tile_pool(name="ps_sc", bufs=2, space="PSUM"))
    ps_y = lp(tc.tile_pool(name="ps_y", bufs=4, space="PSUM"))
    ps_ms = lp(tc.tile_pool(name="ps_ms", bufs=2, space="PSUM"))
    ps_tr = ps_ms  # transposes reuse the dense-phase banks (idle then)

    def layernorm(src, dst_bf):
        """dst_bf = (src - mean)/sqrt(var+eps) over channels, bf16 out."""
        x_bf = stat.tile([128, 4, TLOC], BF16, tag="xbf", name="x_bf")
        nc.vector.tensor_copy(out=x_bf[:], in_=src[:])
        xsq = stat.tile([128, 4, TLOC], BF16, tag="xbf", name="xsq")
        nc.vector.tensor_mul(xsq[:], x_bf[:], x_bf[:])
        ps1 = ps_ms.tile([1, TLOC], F32, tag="ms", name="ps1")
        ps2 = ps_ms.tile([1, TLOC], F32, tag="ms", name="ps2")
        for ks in range(4):
            nc.tensor.matmul(ps1[:], ones4[:, ks], x_bf[:, ks],
                             start=(ks == 0), stop=(ks == 3))
        for ks in range(4):
            nc.tensor.matmul(ps2[:], ones4[:, ks], xsq[:, ks],
                             start=(ks == 0), stop=(ks == 3))
        mu = stat.tile([1, TLOC], F32, tag="mu", name="mu")
        nc.vector.tensor_scalar_mul(mu[:], ps1[:], 1.0 / C)
        var = stat.tile([1, TLOC], F32, tag="var", name="var")
        nc.vector.tensor_scalar_mul(var[:], ps2[:], 1.0 / C)
        musq = stat.tile([1, TLOC], F32, tag="musq", name="musq")
        nc.vector.tensor_mul(musq[:], mu[:], mu[:])
        nc.vector.tensor_sub(var[:], var[:], musq[:])
        nc.scalar.activation(var[:], var[:], mybir.ActivationFunctionType.Sqrt,
                             bias=eps_sb[:], scale=1.0)
        pack = stat.tile([1, 2, TLOC], F32, tag="pack", name="pack")
        nc.vector.reciprocal(pack[:, 1], var[:])
        nc.vector.tensor_mul(pack[:, 0], mu[:], pack[:, 1])
        bc = stat.tile([128, 2, TLOC], F32, tag="bc", name="bc")
        nc.gpsimd.partition_broadcast(bc[:], pack[0:1])
        tmp = stat.tile([128, 4, TLOC], BF16, tag="lnt", name="lntmp")
        nc.vector.tensor_tensor(tmp[:], src[:],
                                bc[:, 1:2].to_broadcast((128, 4, TLOC)),
                                mybir.AluOpType.mult)
        nc.vector.tensor_tensor(dst_bf[:], tmp[:],
                                bc[:, 0:1].to_broadcast((128, 4, TLOC)),
                                mybir.AluOpType.subtract)

    for l in range(L):
        wqkv_sb = wpool.tile([128, 4, 3 * C], BF16, tag="wqkv", name="wqkv_sb")
        nc.sync.dma_start(wqkv_sb[:], wqkvT[l])
        wc_sb = wpool.tile([64, H, C], BF16, tag="wc", name="wc_sb")
        nc.sync.dma_start(wc_sb[:], wcT[l])
        wfc_sb = wpool.tile([128, 4, FF], BF16, tag="wfc", name="wfc_sb")
        nc.sync.dma_start(wfc_sb[:], wfcT[l])
        wpr_sb = wpool.tile([128, 16, C], BF16, tag="wpr", name="wpr_sb")
        nc.sync.dma_start(wpr_sb[:], wprT[l])

        xh1 = acts.tile([128, 4, TLOC], BF16, tag="xh", name="xh1")
        layernorm(xT, xh1)

        # ---- qkv ----------------------------------------------------------
        qT = acts1.tile([128, 4, TLOC], BF16, tag="qT", name="qT")
        kT_loc = acts1.tile([128, 4, TLOC], BF16, tag="kT", name="kT_loc")
        v_aug = acts1.tile([128, 4, H, D + 1], BF16, tag="vaug", name="v_aug")
        nc.vector.memset(v_aug[:, :, :, D:], 1.0)
        W = C + (D + 1) * H  # 1032
        ag_in = dram.tile([TLOC, W], BF16, tag="agin", name="ag_in")
        ag_out = dram.tile([R * TLOC, W], BF16, tag="agout", name="ag_out",
                           addr_space="Shared")
        for blk in range(4, 8):  # k blocks: out [chan, t]
            ps = ps_ms.tile([128, TLOC], F32, tag="ms", name=f"psqk{blk}")
            for ks in range(4):
                nc.tensor.matmul(ps[:],
                                 wqkv_sb[:, ks, blk * 128:(blk + 1) * 128],
                                 xh1[:, ks], start=(ks == 0), stop=(ks == 3))
            nc.vector.tensor_scalar(kT_loc[:, blk % 4], ps[:],
                                    qkvb_sb[:, l, blk:blk + 1], None,
                                    mybir.AluOpType.add)
        for tt in range(4):  # v: out [t, chan]
            ps = ps_ms.tile([128, C], F32, tag="ms", name=f"psv{tt}")
            for ks in range(4):
                nc.tensor.matmul(ps[:], xh1[:, ks, tt * 128:(tt + 1) * 128],
                                 wqkv_sb[:, ks, 2 * C:3 * C],
                                 start=(ks == 0), stop=(ks == 3))
            nc.vector.tensor_scalar(
                v_aug[:, tt, :, 0:D], ps[:].rearrange("p (h d) -> p h d", h=H),
                qkvb_sb[:, l, 8:9], None, mybir.AluOpType.add)

        # ---- KV all-gather (overlaps with q computation) -------------------
        nc.sync.dma_start(
            ag_in[:, 0:C].rearrange("(s p) t -> p s t", p=128), kT_loc[:])
        nc.sync.dma_start(
            ag_in[:, C:W].rearrange("(s p) (h a) -> p s h a", p=128, h=H),
            v_aug[:])
        nc.gpsimd.collective_compute(
            "AllGather", mybir.AluOpType.bypass, replica_groups=rg,
            ins=[ag_in.opt()], outs=[ag_out.opt()])

        for blk in range(4):  # q blocks: out [chan, t]
            ps = ps_ms.tile([128, TLOC], F32, tag="ms", name=f"psq{blk}")
            for ks in range(4):
                nc.tensor.matmul(ps[:],
                                 wqkv_sb[:, ks, blk * 128:(blk + 1) * 128],
                                 xh1[:, ks], start=(ks == 0), stop=(ks == 3))
            nc.vector.tensor_scalar(qT[:, blk], ps[:],
                                    qkvb_sb[:, l, blk:blk + 1], None,
                                    mybir.AluOpType.add)

        yT = acts1.tile([64, H, TLOC], BF16, tag="yT", name="yT")

        for slot in range(2):
            k_half = kvp.tile([128, 4, R, CH], BF16, tag="kh", name="k_half")
            ag_k = ag_out[:, 0:C].rearrange("(r s p) t -> p s r t",
                                            p=128, s=4)
            v_half = kvp.tile([128, R, 2, H, D + 1], BF16, tag="vh",
                              name="v_half")
            ag_v = ag_out[:, C:W].rearrange("(r st p) (h a) -> p r st h a",
                                            p=128, st=4, h=H)
            for r in range(R):
                nc.sync.dma_start(
                    k_half[:, :, r],
                    ag_k[:, :, r, slot * CH:(slot + 1) * CH])
                nc.sync.dma_start(
                    v_half[:, r],
                    ag_v[:, r, 2 * slot:2 * slot + 2])

            den_slot = stat.tile([65, H, CH], F32, tag="dens",
                                 name="den_slot", bufs=1, padded_shape=None)
            for h in range(H):
                po, sub = 64 * (h % 2), h // 2
                q_sl = qT[po:po + 64, sub, slot * CH:(slot + 1) * CH]
                ps_yt = ps_y.tile([D + 1, CH], F32, tag="y", name="ps_yt")
                n_st = 2 * R + 2
                for st in range(n_st):
                    ps_s = ps_sc.tile([128, CH], F32, tag="sc", name="ps_s")
                    if st < 2 * R:
                        lhsT = k_half[po:po + 64, sub, st // 2,
                                      (st % 2) * 128:(st % 2 + 1) * 128]
                    else:
                        m = st - 2 * R
                        lhsT = kT_loc[po:po + 64, sub,
                                      slot * CH + m * 128:
                                      slot * CH + (m + 1) * 128]
                    nc.tensor.matmul(ps_s[:], lhsT, q_sl, start=True,
                                     stop=True)
                    ex = expp.tile([128, CH], BF16, tag="ex", name="ex")
                    if st < 2 * R:
                        nc.scalar.activation(
                            ex[:], ps_s[:], mybir.ActivationFunctionType.Exp,
                            bias=bias_sb[:, slot, st:st + 1], scale=0.125)
                        v_sl = v_half[:, st // 2, st % 2, h, :]
                    else:
                        m = st - 2 * R
                        nc.scalar.activation(
                            ex[:], ps_s[:], mybir.ActivationFunctionType.Exp,
                            bias=zero_sb[:], scale=0.125)
                        nc.vector.tensor_mul(ex[:], ex[:], tri_sb[:, m])
                        v_sl = v_aug[:, 2 * slot + m, h, :]
                    nc.tensor.matmul(ps_yt[:], v_sl, ex[:],
                                     start=(st == 0), stop=(st == n_st - 1))
                # evict unnormalized y (fast) + its denominator row
                nc.vector.tensor_copy(
                    out=yT[:, h, slot * CH:(slot + 1) * CH], in_=ps_yt[0:D, :])
                nc.vector.tensor_copy(out=den_slot[64:65, h],
                                      in_=ps_yt[D:D + 1, :])
            # batched normalization for all 8 heads of this slot
            den8 = stat.tile([H, CH], F32, tag="den8", name="den8")
            nc.sync.dma_start(den8[:], den_slot[64:65])
            rec8 = stat.tile([H, CH], F32, tag="den8", name="rec8")
            nc.vector.reciprocal(rec8[:], den8[:])
            rec1 = stat.tile([1, H, CH], F32, tag="dens", name="rec1", bufs=1)
            nc.sync.dma_start(rec1[0:1], rec8[:])
            den_bc = stat.tile([64, H, CH], F32, tag="denb", name="den_bc", bufs=1)
            nc.gpsimd.partition_broadcast(
                den_bc.rearrange("p h c -> p (h c)"), rec1[0:1])
            for h in range(H):
                sl = yT[:, h, slot * CH:(slot + 1) * CH]
                nc.vector.tensor_tensor(sl, sl, den_bc[:, h],
                                        mybir.AluOpType.mult)

        # ---- attn out proj + residual (per-head K=64 contractions) --------
        for ob in range(4):
            ps = ps_ms.tile([128, TLOC], F32, tag="ms", name=f"psc{ob}")
            for h in range(H):
                nc.tensor.matmul(ps[:],
                                 wc_sb[:, h, ob * 128:(ob + 1) * 128],
                                 yT[:, h], start=(h == 0), stop=(h == H - 1))
            nc.vector.tensor_add(xT[:, ob], xT[:, ob], ps[:])

        # ---- MLP ----------------------------------------------------------
        xh2 = acts.tile([128, 4, TLOC], BF16, tag="xh", name="xh2")
        layernorm(xT, xh2)
        hT = acts1.tile([128, 16, TLOC], BF16, tag="hT", name="hT")
        for fb in range(16):
            ps = ps_ms.tile([128, TLOC], F32, tag="ms", name=f"psf{fb}")
            for ks in range(4):
                nc.tensor.matmul(ps[:], wfc_sb[:, ks, fb * 128:(fb + 1) * 128],
                                 xh2[:, ks], start=(ks == 0), stop=(ks == 3))
            nc.scalar.activation(hT[:, fb], ps[:],
                                 mybir.ActivationFunctionType.Gelu,
                                 bias=fcb_sb[:, l, fb:fb + 1], scale=1.0)
        for ob in range(4):
            ps = ps_ms.tile([128, TLOC], F32, tag="ms", name=f"psp{ob}")
            for ks in range(16):
                nc.tensor.matmul(ps[:], wpr_sb[:, ks, ob * 128:(ob + 1) * 128],
                                 hT[:, ks], start=(ks == 0), stop=(ks == 15))
            nc.vector.tensor_add(xT[:, ob], xT[:, ob], ps[:])

    # ---- final LN + x all-gather -----------------------------------------
    xnf = acts.tile([128, 4, TLOC], BF16, tag="xh", name="xnf")
    layernorm(xT, xnf)
    ag2_in = dram.tile([C, TLOC], BF16, tag="ag2in", name="ag2_in")
    ag2_out = dram.tile([R * C, TLOC], BF16, tag="ag2out", name="ag2_out",
                        addr_space="Shared")
    nc.sync.dma_start(ag2_in.rearrange("(s p) t -> p s t", p=128), xnf[:])
    nc.gpsimd.collective_compute(
        "AllGather", mybir.AluOpType.bypass, replica_groups=rg,
        ins=[ag2_in.opt()], outs=[ag2_out.opt()])

    layer_ctx.close()

    # ---- lm_head ----------------------------------------------------------
    lm = ctx.enter_context(tc.tile_pool(name="lm", bufs=1))
    lmo = ctx.enter_context(tc.tile_pool(name="lmo", bufs=3))
    ps_lm = ctx.enter_context(tc.tile_pool(name="ps_lm", bufs=4, space="PSUM"))

    wte_sb = lm.tile([128, 4, VC], BF16, name="wte_sb")
    nc.sync.dma_start(wte_sb[:], wteT[:])
    xfT = lm.tile([128, 4, R, TLOC], BF16, name="xfT")
    ag2_v = ag2_out.rearrange("(r s p) t -> p s r t", p=128, s=4)
    for r in range(R):
        nc.sync.dma_start(xfT[:, :, r], ag2_v[:, :, r])

    NVC = (VC + 511) // 512  # 13
    for tt in range(R * TLOC // 128):  # 32 token tiles
        lsb = lmo.tile([128, VC], BF16, tag="lsb", name="lsb")
        for vb in range(NVC):
            v0 = vb * 512
            vw = min(512, VC - v0)
            ps = ps_lm.tile([128, 512], F32, tag="lm", name="pslm")
            for ks in range(4):
                nc.tensor.matmul(ps[:, 0:vw],
                                 xfT[:, ks, tt // 4,
                                     (tt % 4) * 128:(tt % 4 + 1) * 128],
                                 wte_sb[:, ks, v0:v0 + vw],
                                 start=(ks == 0), stop=(ks == 3))
            if vb % 2 == 0:
                nc.vector.tensor_copy(out=lsb[:, v0:v0 + vw],
                                      in_=ps[:, 0:vw])
            else:
                nc.scalar.copy(out=lsb[:, v0:v0 + vw], in_=ps[:, 0:vw])
        nc.sync.dma_start(logits[tt * 128:(tt + 1) * 128, :], lsb[:])

    ctx.close()


# ---------------------------------------------------------------------------
# Host side
# ---------------------------------------------------------------------------

_CACHE = {}


def _prep_inputs(idx, wte, wpe, ln1_w, ln1_b, c_attn_w, c_proj_w, ln2_w,
                 ln2_b, fc_w, proj_w, lnf_w, lnf_b):
    idx = np.asarray(idx)
    f32 = lambda a: np.asarray(a, np.float32)
    wte, wpe = f32(wte), f32(wpe)
    ln1_w, ln1_b, ln2_w, ln2_b = f32(ln1_w), f32(ln1_b), f32(ln2_w), f32(ln2_b)
    lnf_w, lnf_b = f32(lnf_w), f32(lnf_b)
    c_attn_w, c_proj_w = f32(c_attn_w), f32(c_proj_w)
    fc_w, proj_w = f32(fc_w), f32(proj_w)

    # biases that cannot be folded per-partition must be zero (they are in
    # the reference: all LayerNorm biases are zeros)
    assert np.all(lnf_b == 0.0), "non-zero lnf_b not supported"

    tok_of_core = []
    for r in range(R):
        pa, pb = _chunks_for_core(r)
        ta = np.arange(pa * CH, (pa + 1) * CH)
        tb = T + np.arange(pb * CH, (pb + 1) * CH)
        tok_of_core.append(np.concatenate([ta, tb]))
    gather_order = np.concatenate(tok_of_core)

    x0 = wte[idx.reshape(-1)] + np.tile(wpe[:T], (B, 1))  # [4096, C]

    wte_pad = np.zeros((VC * R, C), np.float32)
    wte_pad[:V] = wte * lnf_w[None, :]  # fold lnf scale into tied head

    # shared (core-independent) weight prep
    wqkvT = np.zeros((L, 128, 4, 3 * C), bfloat16)
    wcT = np.zeros((L, 64, H, C), bfloat16)
    wfcT = np.zeros((L, 128, 4, FF), bfloat16)
    wprT = np.zeros((L, 128, 16, C), bfloat16)
    qkv_b = np.zeros((L, 128, 9), np.float32)
    fc_b = np.zeros((L, 128, 16), np.float32)
    for l in range(L):
        wq = c_attn_w[l] * ln1_w[l][None, :]
        bq = c_attn_w[l] @ ln1_b[l]  # [3C]
        wqkvT[l] = _to_cpart(wq.T).astype(bfloat16)
        for blk in range(8):
            qkv_b[l, :, blk] = bq[blk * 128:(blk + 1) * 128]
        assert np.allclose(bq[2 * C:], bq[2 * C]), "v bias must be uniform"
        qkv_b[l, :, 8] = bq[2 * C]
        wcT[l] = np.ascontiguousarray(
            c_proj_w[l].T.reshape(H, 64, C).transpose(1, 0, 2)
        ).astype(bfloat16)
        wf = fc_w[l] * ln2_w[l][None, :]
        bf = fc_w[l] @ ln2_b[l]  # [FF]
        wfcT[l] = _to_cpart(wf.T).astype(bfloat16)
        fc_b[l] = bf.reshape(16, 128).T
        wprT[l] = _to_cpart(proj_w[l].T).astype(bfloat16)

    tri = np.zeros((128, 2, CH), bfloat16)
    for m in range(2):
        sp = m * 128 + np.arange(128)
        tri[:, m, :] = (sp[:, None] <= np.arange(CH)[None, :]).astype(
            np.float32)

    in_maps = []
    for r in range(R):
        toks = tok_of_core[r]
        x0T = _to_cpart(np.ascontiguousarray(x0[toks].T))

        ab = np.zeros((128, 2, 16), np.float32)
        pa, pb = _chunks_for_core(r)
        for st in range(16):
            rr = st // 2
            ka, kb = _chunks_for_core(rr)
            ab[:, 0, st] = 0.0 if ka < pa else NEG
            ab[:, 1, st] = 0.0 if kb < pb else NEG

        wteTr = _to_cpart(
            np.ascontiguousarray(wte_pad[r * VC:(r + 1) * VC].T)
        ).astype(bfloat16)

        in_maps.append({
            "x0T": x0T, "wqkvT": wqkvT, "wcT": wcT, "wfcT": wfcT,
            "wprT": wprT, "qkv_b": qkv_b, "fc_b": fc_b,
            "attn_bias": ab, "trimask": tri, "wteT": wteTr,
        })
    return in_maps, gather_order


def kernel(**inputs):
    if "nc" not in _CACHE:
        _CACHE["nc"] = build_program()
    nc = _CACHE["nc"]

    in_maps, gather_order = _prep_inputs(**inputs)
    res = run_bass_kernel_spmd(nc, in_maps, core_ids=list(range(R)))
    _CACHE["res"] = res  # exec_time_ns etc when BASS_TRACE=1
    outs = [np.asarray(res.results[r]["logits"]) for r in range(R)]
    full = np.concatenate(outs, axis=1)  # [4096, R*VC] gathered row order
    inv = np.empty(B * T, np.int64)
    inv[gather_order] = np.arange(B * T)
    full = full[inv][:, :V].astype(np.float32)
    return full.reshape(B, T, V)


# revision 32
# speedup vs baseline: 1.2229x; 1.2229x over previous
"""MiniGPT forward pass on 8 Trainium2 NeuronCores (Bass/Tile).

Sharding: token-sharded transformer (core r owns batch-0 chunk r and batch-1
chunk 7-r, 256 tokens each), per-layer KV AllGather, attention over the full
gathered key set with per-core causal chunk bias folded into the exp bias,
and the on-diagonal 256x256 block computed from local K/V with a constant
triangular mask.  lm_head is vocab-sharded (tied wte) after a final
AllGather of the normalized activations.  Matmuls run in bf16 with fp32
PSUM accumulation; logits are written in bf16 and upcast on the host.

Device layout: the residual stream is channel-major ("transposed"):
xT[p, sub, t] with channel c = sub*128 + p; every matmul consumes/produces
channel-major tensors so the residual never needs an on-device transpose.
"""

import sys

sys.path.insert(0, "/opt/trn_rl_repo")

from contextlib import ExitStack

import numpy as np
from ml_dtypes import bfloat16

import concourse.bass as bass  # noqa: F401
import concourse.tile as tile
from concourse import bacc, mybir
from concourse.bass_utils import run_bass_kernel_spmd
from concourse.masks import make_identity

V, BLK, L, H, C, FF = 50257, 2048, 6, 8, 512, 2048
D = C // H  # 64
B, T = 2, 2048
R = 8
TLOC = 512  # tokens per core
CH = 256  # chunk size
VC = 6283  # vocab slice per core (8*6283 >= 50257)
NEG = -30.0

F32 = mybir.dt.float32
BF16 = mybir.dt.bfloat16


def _to_cpart(a):
    """[C, N] -> [128, C//128, N] with c = sub*128 + p."""
    Cdim, N = a.shape
    return np.ascontiguousarray(a.reshape(Cdim // 128, 128, N).transpose(1, 0, 2))


def _chunks_for_core(r):
    return r, 7 - r


# ---------------------------------------------------------------------------
# Device program
# ---------------------------------------------------------------------------

def build_program():
    nc = bacc.Bacc("TRN2", target_bir_lowering=False, debug=False,
                   num_devices=R)

    def din(name, shape, dt):
        return nc.dram_tensor(name, shape, dt, kind="ExternalInput").ap()

    io = {
        "x0T": din("x0T", [128, 4, TLOC], F32),
        "wqkvT": din("wqkvT", [L, 128, 4, 3 * C], BF16),
        "wcT": din("wcT", [L, 128, 4, C], BF16),
        "wfcT": din("wfcT", [L, 128, 4, FF], BF16),
        "wprT": din("wprT", [L, 128, 16, C], BF16),
        "qkv_b": din("qkv_b", [L, 128, 9], F32),
        "fc_b": din("fc_b", [L, 128, 16], F32),
        "attn_bias": din("attn_bias", [128, 2, 16], F32),
        "trimask": din("trimask", [128, 2, CH], BF16),
        "wteT": din("wteT", [128, 4, VC], BF16),
        "logits": nc.dram_tensor("logits", [4096, VC], BF16,
                                 kind="ExternalOutput").ap(),
        "rg": [list(range(R))],
    }

    with tile.TileContext(nc) as tc:
        _build_body(nc, tc, io)
    nc.compile()
    return nc


def _build_body(nc, tc, io):
    x0T, wqkvT, wcT, wfcT, wprT = (io["x0T"], io["wqkvT"], io["wcT"],
                                   io["wfcT"], io["wprT"])
    qkv_b, fc_b = io["qkv_b"], io["fc_b"]
    attn_bias, trimask = io["attn_bias"], io["trimask"]
    wteT, logits, rg = io["wteT"], io["logits"], io["rg"]

    ctx = ExitStack()
    const = ctx.enter_context(tc.tile_pool(name="const", bufs=1))
    dram = ctx.enter_context(tc.tile_pool(name="dram", bufs=1, space="DRAM"))

    xT = const.tile([128, 4, TLOC], F32)
    nc.sync.dma_start(xT[:], x0T[:])

    ident = const.tile([128, 128], BF16)
    make_identity(nc, ident)
    ones4 = const.tile([128, 4, 1], BF16)
    nc.vector.memset(ones4[:], 1.0)
    bias_sb = const.tile([128, 2, 16], F32)
    nc.sync.dma_start(bias_sb[:], attn_bias[:])
    tri_sb = const.tile([128, 2, CH], BF16)
    nc.sync.dma_start(tri_sb[:], trimask[:])
    qkvb_sb = const.tile([128, L, 9], F32)
    nc.sync.dma_start(qkvb_sb[:], qkv_b.rearrange("l p n -> p l n"))
    fcb_sb = const.tile([128, L, 16], F32)
    nc.sync.dma_start(fcb_sb[:], fc_b.rearrange("l p n -> p l n"))
    eps_sb = const.tile([1, 1], F32)
    nc.vector.memset(eps_sb[:], 1e-5)
    zero_sb = const.tile([128, 1], F32)
    nc.vector.memset(zero_sb[:], 0.0)

    layer_ctx = ExitStack()
    lp = layer_ctx.enter_context
    wpool = lp(tc.tile_pool(name="wpool", bufs=1))
    acts = lp(tc.tile_pool(name="acts", bufs=2))   # xh tiles
    acts1 = lp(tc.tile_pool(name="acts1", bufs=1))  # qT/kT/vaug/yT/hT
    kvp = lp(tc.tile_pool(name="kvp", bufs=1))
    expp = lp(tc.tile_pool(name="expp", bufs=4))
    stat = lp(tc.tile_pool(name="stat", bufs=2))
    ps_sc = lp(tc.tile_pool(name="ps_sc", bufs=2, space="PSUM"))
    ps_y = lp(tc.tile_pool(name="ps_y", bufs=4, space="PSUM"))
    ps_ms = lp(tc.tile_pool(name="ps_ms", bufs=2, space="PSUM"))
    ps_tr = ps_ms  # transposes reuse the dense-phase banks (idle then)

    def layernorm(src, dst_bf):
        """dst_bf = (src - mean)/sqrt(var+eps) over channels, bf16 out."""
        x_bf = stat.tile([128, 4, TLOC], BF16, tag="xbf", name="x_bf")
        nc.vector.tensor_copy(out=x_bf[:], in_=src[:])
        xsq = stat.tile([128, 4, TLOC], BF16, tag="xbf", name="xsq")
        nc.vector.tensor_mul(xsq[:], x_bf[:], x_bf[:])
        ps1 = ps_ms.tile([1, TLOC], F32, tag="ms", name="ps1")
        ps2 = ps_ms.tile([1, TLOC], F32, tag="ms", name="ps2")
        for ks in range(4):
            nc.tensor.matmul(ps1[:], ones4[:, ks], x_bf[:, ks],
                             start=(ks == 0), stop=(ks == 3))
        for ks in range(4):
            nc.tensor.matmul(ps2[:], ones4[:, ks], xsq[:, ks],
                             start=(ks == 0), stop=(ks == 3))
        mu = stat.tile([1, TLOC], F32, tag="mu", name="mu")
        nc.vector.tensor_scalar_mul(mu[:], ps1[:], 1.0 / C)
        var = stat.tile([1, TLOC], F32, tag="var", name="var")
        nc.vector.tensor_scalar_mul(var[:], ps2[:], 1.0 / C)
        musq = stat.tile([1, TLOC], F32, tag="musq", name="musq")
        nc.vector.tensor_mul(musq[:], mu[:], mu[:])
        nc.vector.tensor_sub(var[:], var[:], musq[:])
        nc.scalar.activation(var[:], var[:], mybir.ActivationFunctionType.Sqrt,
                             bias=eps_sb[:], scale=1.0)
        pack = stat.tile([1, 2, TLOC], F32, tag="pack", name="pack")
        nc.vector.reciprocal(pack[:, 1], var[:])
        nc.vector.tensor_mul(pack[:, 0], mu[:], pack[:, 1])
        bc = stat.tile([128, 2, TLOC], F32, tag="bc", name="bc")
        nc.gpsimd.partition_broadcast(bc[:], pack[0:1])
        tmp = stat.tile([128, 4, TLOC], BF16, tag="lnt", name="lntmp")
        nc.vector.tensor_tensor(tmp[:], src[:],
                                bc[:, 1:2].to_broadcast((128, 4, TLOC)),
                                mybir.AluOpType.mult)
        nc.vector.tensor_tensor(dst_bf[:], tmp[:],
                                bc[:, 0:1].to_broadcast((128, 4, TLOC)),
                                mybir.AluOpType.subtract)

    for l in range(L):
        wqkv_sb = wpool.tile([128, 4, 3 * C], BF16, tag="wqkv", name="wqkv_sb")
        nc.sync.dma_start(wqkv_sb[:], wqkvT[l])
        wc_sb = wpool.tile([128, 4, C], BF16, tag="wc", name="wc_sb")
        nc.sync.dma_start(wc_sb[:], wcT[l])
        wfc_sb = wpool.tile([128, 4, FF], BF16, tag="wfc", name="wfc_sb")
        nc.sync.dma_start(wfc_sb[:], wfcT[l])
        wpr_sb = wpool.tile([128, 16, C], BF16, tag="wpr", name="wpr_sb")
        nc.sync.dma_start(wpr_sb[:], wprT[l])

        xh1 = acts.tile([128, 4, TLOC], BF16, tag="xh", name="xh1")
        layernorm(xT, xh1)

        # ---- qkv ----------------------------------------------------------
        qT = acts1.tile([128, 4, TLOC], BF16, tag="qT", name="qT")
        kT_loc = acts1.tile([128, 4, TLOC], BF16, tag="kT", name="kT_loc")
        v_aug = acts1.tile([128, 4, H, D + 1], BF16, tag="vaug", name="v_aug")
        nc.vector.memset(v_aug[:, :, :, D:], 1.0)
        W = C + (D + 1) * H  # 1032
        ag_in = dram.tile([TLOC, W], BF16, tag="agin", name="ag_in")
        ag_out = dram.tile([R * TLOC, W], BF16, tag="agout", name="ag_out",
                           addr_space="Shared")
        for blk in range(4, 8):  # k blocks: out [chan, t]
            ps = ps_ms.tile([128, TLOC], F32, tag="ms", name=f"psqk{blk}")
            for ks in range(4):
                nc.tensor.matmul(ps[:],
                                 wqkv_sb[:, ks, blk * 128:(blk + 1) * 128],
                                 xh1[:, ks], start=(ks == 0), stop=(ks == 3))
            nc.vector.tensor_scalar(kT_loc[:, blk % 4], ps[:],
                                    qkvb_sb[:, l, blk:blk + 1], None,
                                    mybir.AluOpType.add)
        for tt in range(4):  # v: out [t, chan]
            ps = ps_ms.tile([128, C], F32, tag="ms", name=f"psv{tt}")
            for ks in range(4):
                nc.tensor.matmul(ps[:], xh1[:, ks, tt * 128:(tt + 1) * 128],
                                 wqkv_sb[:, ks, 2 * C:3 * C],
                                 start=(ks == 0), stop=(ks == 3))
            nc.vector.tensor_scalar(
                v_aug[:, tt, :, 0:D], ps[:].rearrange("p (h d) -> p h d", h=H),
                qkvb_sb[:, l, 8:9], None, mybir.AluOpType.add)

        # ---- KV all-gather (overlaps with q computation) -------------------
        nc.sync.dma_start(
            ag_in[:, 0:C].rearrange("(s p) t -> p s t", p=128), kT_loc[:])
        nc.sync.dma_start(
            ag_in[:, C:W].rearrange("(s p) (h a) -> p s h a", p=128, h=H),
            v_aug[:])
        nc.gpsimd.collective_compute(
            "AllGather", mybir.AluOpType.bypass, replica_groups=rg,
            ins=[ag_in.opt()], outs=[ag_out.opt()])

        for blk in range(4):  # q blocks: out [chan, t]
            ps = ps_ms.tile([128, TLOC], F32, tag="ms", name=f"psq{blk}")
            for ks in range(4):
                nc.tensor.matmul(ps[:],
                                 wqkv_sb[:, ks, blk * 128:(blk + 1) * 128],
                                 xh1[:, ks], start=(ks == 0), stop=(ks == 3))
            nc.vector.tensor_scalar(qT[:, blk], ps[:],
                                    qkvb_sb[:, l, blk:blk + 1], None,
                                    mybir.AluOpType.add)

        yT = acts1.tile([128, 4, TLOC], BF16, tag="yT", name="yT")

        # diagonal score tiles from local K (computed while the AG runs)
        exd = acts1.tile([128, 2, H, 2, CH], BF16, tag="exd", name="exd")
        for slot in range(2):
            for h in range(H):
                po, sub = 64 * (h % 2), h // 2
                q_sl = qT[po:po + 64, sub, slot * CH:(slot + 1) * CH]
                for m in range(2):
                    ps_s = ps_sc.tile([128, CH], F32, tag="sc", name="ps_sd")
                    lhsT = kT_loc[po:po + 64, sub,
                                  slot * CH + m * 128:slot * CH + (m + 1) * 128]
                    nc.tensor.matmul(ps_s[:], lhsT, q_sl, start=True,
                                     stop=True)
                    nc.scalar.activation(
                        exd[:, slot, h, m], ps_s[:],
                        mybir.ActivationFunctionType.Exp,
                        bias=zero_sb[:], scale=0.125)
                    nc.vector.tensor_mul(exd[:, slot, h, m],
                                         exd[:, slot, h, m], tri_sb[:, m])

        for slot in range(2):
            k_half = kvp.tile([128, 4, R, CH], BF16, tag="kh", name="k_half")
            ag_k = ag_out[:, 0:C].rearrange("(r s p) t -> p s r t",
                                            p=128, s=4)
            v_half = kvp.tile([128, R, 2, H, D + 1], BF16, tag="vh",
                              name="v_half")
            ag_v = ag_out[:, C:W].rearrange("(r st p) (h a) -> p r st h a",
                                            p=128, st=4, h=H)
            for r in range(R):
                nc.sync.dma_start(
                    k_half[:, :, r],
                    ag_k[:, :, r, slot * CH:(slot + 1) * CH])
                nc.sync.dma_start(
                    v_half[:, r],
                    ag_v[:, r, 2 * slot:2 * slot + 2])

            for h in range(H):
                po, sub = 64 * (h % 2), h // 2
                q_sl = qT[po:po + 64, sub, slot * CH:(slot + 1) * CH]
                ps_ya = ps_y.tile([128, D + 1], F32, tag="y", name="ps_ya")
                ps_yb = ps_y.tile([128, D + 1], F32, tag="y", name="ps_yb")
                n_st = 2 * R + 2
                for st in range(n_st):
                    if st < 2 * R:
                        ps_s = ps_sc.tile([128, CH], F32, tag="sc",
                                          name="ps_s")
                        lhsT = k_half[po:po + 64, sub, st // 2,
                                      (st % 2) * 128:(st % 2 + 1) * 128]
                        nc.tensor.matmul(ps_s[:], lhsT, q_sl, start=True,
                                         stop=True)
                        ex = expp.tile([128, CH], BF16, tag="ex", name="ex")
                        nc.scalar.activation(
                            ex[:], ps_s[:], mybir.ActivationFunctionType.Exp,
                            bias=bias_sb[:, slot, st:st + 1], scale=0.125)
                        v_sl = v_half[:, st // 2, st % 2, h, :]
                    else:
                        m = st - 2 * R
                        ex = exd[:, slot, h, m]
                        v_sl = v_aug[:, 2 * slot + m, h, :]
                    nc.tensor.matmul(ps_ya[:], ex[:, 0:128], v_sl,
                                     start=(st == 0), stop=(st == n_st - 1))
                    nc.tensor.matmul(ps_yb[:], ex[:, 128:256], v_sl,
                                     start=(st == 0), stop=(st == n_st - 1))
                for tb, ps_yt in enumerate((ps_ya, ps_yb)):
                    den = stat.tile([128, 1], F32, tag="den", name="den")
                    nc.vector.reciprocal(den[:], ps_yt[:, D:D + 1])
                    ysc = expp.tile([128, D], BF16, tag="ysc", name="ysc")
                    nc.vector.tensor_scalar(ysc[:], ps_yt[:, 0:D], den[:],
                                            None, mybir.AluOpType.mult)
                    pst = ps_ms.tile([64, 128], BF16, tag="ms", name="pst")
                    nc.tensor.transpose(pst[:], ysc[:], ident[:])
                    nc.vector.tensor_copy(
                        out=yT[po:po + 64, sub,
                               slot * CH + tb * 128:slot * CH + (tb + 1) * 128],
                        in_=pst[:])

        # ---- attn out proj + residual ------------------------------------
        for ob in range(4):
            ps = ps_ms.tile([128, TLOC], F32, tag="ms", name=f"psc{ob}")
            for ks in range(4):
                nc.tensor.matmul(ps[:], wc_sb[:, ks, ob * 128:(ob + 1) * 128],
                                 yT[:, ks], start=(ks == 0), stop=(ks == 3))
            nc.vector.tensor_add(xT[:, ob], xT[:, ob], ps[:])

        # ---- MLP ----------------------------------------------------------
        xh2 = acts.tile([128, 4, TLOC], BF16, tag="xh", name="xh2")
        layernorm(xT, xh2)
        hT = acts1.tile([128, 16, TLOC], BF16, tag="hT", name="hT")
        for fb in range(16):
            ps = ps_ms.tile([128, TLOC], F32, tag="ms", name=f"psf{fb}")
            for ks in range(4):
                nc.tensor.matmul(ps[:], wfc_sb[:, ks, fb * 128:(fb + 1) * 128],
                                 xh2[:, ks], start=(ks == 0), stop=(ks == 3))
            nc.scalar.activation(hT[:, fb], ps[:],
                                 mybir.ActivationFunctionType.Gelu,
                                 bias=fcb_sb[:, l, fb:fb + 1], scale=1.0)
        for ob in range(4):
            ps = ps_ms.tile([128, TLOC], F32, tag="ms", name=f"psp{ob}")
            for ks in range(16):
                nc.tensor.matmul(ps[:], wpr_sb[:, ks, ob * 128:(ob + 1) * 128],
                                 hT[:, ks], start=(ks == 0), stop=(ks == 15))
            nc.vector.tensor_add(xT[:, ob], xT[:, ob], ps[:])

    # ---- final LN + x all-gather -----------------------------------------
    xnf = acts.tile([128, 4, TLOC], BF16, tag="xh", name="xnf")
    layernorm(xT, xnf)
    ag2_in = dram.tile([C, TLOC], BF16, tag="ag2in", name="ag2_in")
    ag2_out = dram.tile([R * C, TLOC], BF16, tag="ag2out", name="ag2_out",
                        addr_space="Shared")
    nc.sync.dma_start(ag2_in.rearrange("(s p) t -> p s t", p=128), xnf[:])
    nc.gpsimd.collective_compute(
        "AllGather", mybir.AluOpType.bypass, replica_groups=rg,
        ins=[ag2_in.opt()], outs=[ag2_out.opt()])

    layer_ctx.close()

    # ---- lm_head ----------------------------------------------------------
    lm = ctx.enter_context(tc.tile_pool(name="lm", bufs=1))
    lmo = ctx.enter_context(tc.tile_pool(name="lmo", bufs=3))
    ps_lm = ctx.enter_context(tc.tile_pool(name="ps_lm", bufs=4, space="PSUM"))

    wte_sb = lm.tile([128, 4, VC], BF16, name="wte_sb")
    nc.sync.dma_start(wte_sb[:], wteT[:])
    xfT = lm.tile([128, 4, R, TLOC], BF16, name="xfT")
    ag2_v = ag2_out.rearrange("(r s p) t -> p s r t", p=128, s=4)
    for r in range(R):
        nc.sync.dma_start(xfT[:, :, r], ag2_v[:, :, r])

    NVC = (VC + 511) // 512  # 13
    for tt in range(R * TLOC // 128):  # 32 token tiles
        lsb = lmo.tile([128, VC], BF16, tag="lsb", name="lsb")
        for vb in range(NVC):
            v0 = vb * 512
            vw = min(512, VC - v0)
            ps = ps_lm.tile([128, 512], F32, tag="lm", name="pslm")
            for ks in range(4):
                nc.tensor.matmul(ps[:, 0:vw],
                                 xfT[:, ks, tt // 4,
                                     (tt % 4) * 128:(tt % 4 + 1) * 128],
                                 wte_sb[:, ks, v0:v0 + vw],
                                 start=(ks == 0), stop=(ks == 3))
            if vb % 2 == 0:
                nc.vector.tensor_copy(out=lsb[:, v0:v0 + vw],
                                      in_=ps[:, 0:vw])
            else:
                nc.scalar.copy(out=lsb[:, v0:v0 + vw], in_=ps[:, 0:vw])
        nc.sync.dma_start(logits[tt * 128:(tt + 1) * 128, :], lsb[:])

    ctx.close()


# ---------------------------------------------------------------------------
# Host side
# ---------------------------------------------------------------------------

_CACHE = {}


def _prep_inputs(idx, wte, wpe, ln1_w, ln1_b, c_attn_w, c_proj_w, ln2_w,
                 ln2_b, fc_w, proj_w, lnf_w, lnf_b):
    idx = np.asarray(idx)
    f32 = lambda a: np.asarray(a, np.float32)
    wte, wpe = f32(wte), f32(wpe)
    ln1_w, ln1_b, ln2_w, ln2_b = f32(ln1_w), f32(ln1_b), f32(ln2_w), f32(ln2_b)
    lnf_w, lnf_b = f32(lnf_w), f32(lnf_b)
    c_attn_w, c_proj_w = f32(c_attn_w), f32(c_proj_w)
    fc_w, proj_w = f32(fc_w), f32(proj_w)

    # biases that cannot be folded per-partition must be zero (they are in
    # the reference: all LayerNorm biases are zeros)
    assert np.all(lnf_b == 0.0), "non-zero lnf_b not supported"

    tok_of_core = []
    for r in range(R):
        pa, pb = _chunks_for_core(r)
        ta = np.arange(pa * CH, (pa + 1) * CH)
        tb = T + np.arange(pb * CH, (pb + 1) * CH)
        tok_of_core.append(np.concatenate([ta, tb]))
    gather_order = np.concatenate(tok_of_core)

    x0 = wte[idx.reshape(-1)] + np.tile(wpe[:T], (B, 1))  # [4096, C]

    wte_pad = np.zeros((VC * R, C), np.float32)
    wte_pad[:V] = wte * lnf_w[None, :]  # fold lnf scale into tied head

    # shared (core-independent) weight prep
    wqkvT = np.zeros((L, 128, 4, 3 * C), bfloat16)
    wcT = np.zeros((L, 128, 4, C), bfloat16)
    wfcT = np.zeros((L, 128, 4, FF), bfloat16)
    wprT = np.zeros((L, 128, 16, C), bfloat16)
    qkv_b = np.zeros((L, 128, 9), np.float32)
    fc_b = np.zeros((L, 128, 16), np.float32)
    for l in range(L):
        wq = c_attn_w[l] * ln1_w[l][None, :]
        bq = c_attn_w[l] @ ln1_b[l]  # [3C]
        wqkvT[l] = _to_cpart(wq.T).astype(bfloat16)
        for blk in range(8):
            qkv_b[l, :, blk] = bq[blk * 128:(blk + 1) * 128]
        assert np.allclose(bq[2 * C:], bq[2 * C]), "v bias must be uniform"
        qkv_b[l, :, 8] = bq[2 * C]
        wcT[l] = _to_cpart(c_proj_w[l].T).astype(bfloat16)
        wf = fc_w[l] * ln2_w[l][None, :]
        bf = fc_w[l] @ ln2_b[l]  # [FF]
        wfcT[l] = _to_cpart(wf.T).astype(bfloat16)
        fc_b[l] = bf.reshape(16, 128).T
        wprT[l] = _to_cpart(proj_w[l].T).astype(bfloat16)

    tri = np.zeros((128, 2, CH), bfloat16)
    for m in range(2):
        sp = m * 128 + np.arange(128)
        tri[:, m, :] = (sp[:, None] <= np.arange(CH)[None, :]).astype(
            np.float32)

    in_maps = []
    for r in range(R):
        toks = tok_of_core[r]
        x0T = _to_cpart(np.ascontiguousarray(x0[toks].T))

        ab = np.zeros((128, 2, 16), np.float32)
        pa, pb = _chunks_for_core(r)
        for st in range(16):
            rr = st // 2
            ka, kb = _chunks_for_core(rr)
            ab[:, 0, st] = 0.0 if ka < pa else NEG
            ab[:, 1, st] = 0.0 if kb < pb else NEG

        wteTr = _to_cpart(
            np.ascontiguousarray(wte_pad[r * VC:(r + 1) * VC].T)
        ).astype(bfloat16)

        in_maps.append({
            "x0T": x0T, "wqkvT": wqkvT, "wcT": wcT, "wfcT": wfcT,
            "wprT": wprT, "qkv_b": qkv_b, "fc_b": fc_b,
            "attn_bias": ab, "trimask": tri, "wteT": wteTr,
        })
    return in_maps, gather_order


def kernel(**inputs):
    if "nc" not in _CACHE:
        _CACHE["nc"] = build_program()
    nc = _CACHE["nc"]

    in_maps, gather_order = _prep_inputs(**inputs)
    res = run_bass_kernel_spmd(nc, in_maps, core_ids=list(range(R)))
    _CACHE["res"] = res  # exec_time_ns etc when BASS_TRACE=1
    outs = [np.asarray(res.results[r]["logits"]) for r in range(R)]
    full = np.concatenate(outs, axis=1)  # [4096, R*VC] gathered row order
    inv = np.empty(B * T, np.int64)
    inv[gather_order] = np.arange(B * T)
    full = full[inv][:, :V].astype(np.float32)
    return full.reshape(B, T, V)


# revision 34
# speedup vs baseline: 1.5062x; 1.2317x over previous
"""MiniGPT forward pass on 8 Trainium2 NeuronCores (Bass/Tile).

Sharding: token-sharded transformer (core r owns batch-0 chunk r and batch-1
chunk 7-r, 256 tokens each), per-layer KV AllGather, attention over the full
gathered key set with per-core causal chunk bias folded into the exp bias,
and the on-diagonal 256x256 block computed from local K/V with a constant
triangular mask.  lm_head is vocab-sharded (tied wte) after a final
AllGather of the normalized activations.  Matmuls run in bf16 with fp32
PSUM accumulation; logits are written in bf16 and upcast on the host.

Device layout: the residual stream is channel-major ("transposed"):
xT[p, sub, t] with channel c = sub*128 + p; every matmul consumes/produces
channel-major tensors so the residual never needs an on-device transpose.
"""

import sys

sys.path.insert(0, "/opt/trn_rl_repo")

from contextlib import ExitStack

import numpy as np
from ml_dtypes import bfloat16

import concourse.bass as bass  # noqa: F401
import concourse.tile as tile
from concourse import bacc, mybir
from concourse.bass_utils import run_bass_kernel_spmd
from concourse.masks import make_identity

V, BLK, L, H, C, FF = 50257, 2048, 6, 8, 512, 2048
D = C // H  # 64
B, T = 2, 2048
R = 8
TLOC = 512  # tokens per core
CH = 256  # chunk size
VC = 6283  # vocab slice per core (8*6283 >= 50257)
NEG = -30.0

F32 = mybir.dt.float32
BF16 = mybir.dt.bfloat16


def _to_cpart(a):
    """[C, N] -> [128, C//128, N] with c = sub*128 + p."""
    Cdim, N = a.shape
    return np.ascontiguousarray(a.reshape(Cdim // 128, 128, N).transpose(1, 0, 2))


def _chunks_for_core(r):
    return r, 7 - r


# ---------------------------------------------------------------------------
# Device program
# ---------------------------------------------------------------------------

def build_program():
    nc = bacc.Bacc("TRN2", target_bir_lowering=False, debug=False,
                   num_devices=R)

    def din(name, shape, dt):
        return nc.dram_tensor(name, shape, dt, kind="ExternalInput").ap()

    io = {
        "x0T": din("x0T", [128, 4, TLOC], F32),
        "wqkvT": din("wqkvT", [L, 128, 4, 3 * C], BF16),
        "wcT": din("wcT", [L, 128, 4, C], BF16),
        "wfcT": din("wfcT", [L, 128, 4, FF], BF16),
        "wprT": din("wprT", [L, 128, 16, C], BF16),
        "qkv_b": din("qkv_b", [L, 128, 9], F32),
        "fc_b": din("fc_b", [L, 128, 16], F32),
        "attn_bias": din("attn_bias", [128, 2, 14], F32),
        "gath_off": din("gath_off", [1, 32], mybir.dt.int32),
        "trimask": din("trimask", [128, 2, CH], BF16),
        "wteT": din("wteT", [128, 4, VC], BF16),
        "logits": nc.dram_tensor("logits", [4096, VC], BF16,
                                 kind="ExternalOutput").ap(),
        "rg": [list(range(R))],
    }

    with tile.TileContext(nc) as tc:
        _build_body(nc, tc, io)
    nc.compile()
    return nc


def _build_body(nc, tc, io):
    x0T, wqkvT, wcT, wfcT, wprT = (io["x0T"], io["wqkvT"], io["wcT"],
                                   io["wfcT"], io["wprT"])
    qkv_b, fc_b = io["qkv_b"], io["fc_b"]
    attn_bias, trimask = io["attn_bias"], io["trimask"]
    gath_off = io["gath_off"]
    wteT, logits, rg = io["wteT"], io["logits"], io["rg"]

    ctx = ExitStack()
    const = ctx.enter_context(tc.tile_pool(name="const", bufs=1))
    dram = ctx.enter_context(tc.tile_pool(name="dram", bufs=1, space="DRAM"))

    xT = const.tile([128, 4, TLOC], F32)
    nc.sync.dma_start(xT[:], x0T[:])

    ident = const.tile([128, 128], BF16)
    make_identity(nc, ident)
    ones4 = const.tile([128, 4, 1], BF16)
    nc.vector.memset(ones4[:], 1.0)
    bias_sb = const.tile([128, 2, 14], F32)
    nc.sync.dma_start(bias_sb[:], attn_bias[:])
    tri_sb = const.tile([128, 2, CH], BF16)
    nc.sync.dma_start(tri_sb[:], trimask[:])
    qkvb_sb = const.tile([128, L, 9], F32)
    nc.sync.dma_start(qkvb_sb[:], qkv_b.rearrange("l p n -> p l n"))
    fcb_sb = const.tile([128, L, 16], F32)
    nc.sync.dma_start(fcb_sb[:], fc_b.rearrange("l p n -> p l n"))
    eps_sb = const.tile([1, 1], F32)
    nc.vector.memset(eps_sb[:], 1e-5)
    zero_sb = const.tile([128, 1], F32)
    nc.vector.memset(zero_sb[:], 0.0)
    goff_sb = const.tile([1, 32], mybir.dt.int32)
    nc.sync.dma_start(goff_sb[:], gath_off[:])
    goff = []
    for i in range(30):
        reg = nc.alloc_register(mybir.EngineType.Pool, name=f"goff{i}")
        nc.reg_load(reg, goff_sb[0:1, i:i + 1])
        goff.append(nc.gpsimd.snap(reg))
    KLO, KHI = 3, 7

    layer_ctx = ExitStack()
    lp = layer_ctx.enter_context
    wpool = lp(tc.tile_pool(name="wpool", bufs=1))
    acts = lp(tc.tile_pool(name="acts", bufs=2))   # xh tiles
    acts1 = lp(tc.tile_pool(name="acts1", bufs=1))  # qT/kT/vaug/yT/hT
    kvp = lp(tc.tile_pool(name="kvp", bufs=1))
    expp = lp(tc.tile_pool(name="expp", bufs=4))
    stat = lp(tc.tile_pool(name="stat", bufs=2))
    ps_sc = lp(tc.tile_pool(name="ps_sc", bufs=2, space="PSUM"))
    ps_y = lp(tc.tile_pool(name="ps_y", bufs=4, space="PSUM"))
    ps_ms = lp(tc.tile_pool(name="ps_ms", bufs=2, space="PSUM"))
    ps_tr = ps_ms  # transposes reuse the dense-phase banks (idle then)

    def layernorm(src, dst_bf):
        """dst_bf = (src - mean)/sqrt(var+eps) over channels, bf16 out."""
        x_bf = stat.tile([128, 4, TLOC], BF16, tag="xbf", name="x_bf")
        nc.vector.tensor_copy(out=x_bf[:], in_=src[:])
        xsq = stat.tile([128, 4, TLOC], BF16, tag="xbf", name="xsq")
        nc.vector.tensor_mul(xsq[:], x_bf[:], x_bf[:])
        ps1 = ps_ms.tile([1, TLOC], F32, tag="ms", name="ps1")
        ps2 = ps_ms.tile([1, TLOC], F32, tag="ms", name="ps2")
        for ks in range(4):
            nc.tensor.matmul(ps1[:], ones4[:, ks], x_bf[:, ks],
                             start=(ks == 0), stop=(ks == 3))
        for ks in range(4):
            nc.tensor.matmul(ps2[:], ones4[:, ks], xsq[:, ks],
                             start=(ks == 0), stop=(ks == 3))
        mu = stat.tile([1, TLOC], F32, tag="mu", name="mu")
        nc.vector.tensor_scalar_mul(mu[:], ps1[:], 1.0 / C)
        var = stat.tile([1, TLOC], F32, tag="var", name="var")
        nc.vector.tensor_scalar_mul(var[:], ps2[:], 1.0 / C)
        musq = stat.tile([1, TLOC], F32, tag="musq", name="musq")
        nc.vector.tensor_mul(musq[:], mu[:], mu[:])
        nc.vector.tensor_sub(var[:], var[:], musq[:])
        nc.scalar.activation(var[:], var[:], mybir.ActivationFunctionType.Sqrt,
                             bias=eps_sb[:], scale=1.0)
        pack = stat.tile([1, 2, TLOC], F32, tag="pack", name="pack")
        nc.vector.reciprocal(pack[:, 1], var[:])
        nc.vector.tensor_mul(pack[:, 0], mu[:], pack[:, 1])
        bc = stat.tile([128, 2, TLOC], F32, tag="bc", name="bc")
        nc.gpsimd.partition_broadcast(bc[:], pack[0:1])
        tmp = stat.tile([128, 4, TLOC], BF16, tag="lnt", name="lntmp")
        nc.vector.tensor_tensor(tmp[:], src[:],
                                bc[:, 1:2].to_broadcast((128, 4, TLOC)),
                                mybir.AluOpType.mult)
        nc.vector.tensor_tensor(dst_bf[:], tmp[:],
                                bc[:, 0:1].to_broadcast((128, 4, TLOC)),
                                mybir.AluOpType.subtract)

    for l in range(L):
        wqkv_sb = wpool.tile([128, 4, 3 * C], BF16, tag="wqkv", name="wqkv_sb")
        nc.sync.dma_start(wqkv_sb[:], wqkvT[l])
        wc_sb = wpool.tile([128, 4, C], BF16, tag="wc", name="wc_sb")
        nc.sync.dma_start(wc_sb[:], wcT[l])
        wfc_sb = wpool.tile([128, 4, FF], BF16, tag="wfc", name="wfc_sb")
        nc.sync.dma_start(wfc_sb[:], wfcT[l])
        wpr_sb = wpool.tile([128, 16, C], BF16, tag="wpr", name="wpr_sb")
        nc.sync.dma_start(wpr_sb[:], wprT[l])

        xh1 = acts.tile([128, 4, TLOC], BF16, tag="xh", name="xh1")
        layernorm(xT, xh1)

        # ---- qkv ----------------------------------------------------------
        qT = acts1.tile([128, 4, TLOC], BF16, tag="qT", name="qT")
        kT_loc = acts1.tile([128, 4, TLOC], BF16, tag="kT", name="kT_loc")
        v_aug = acts1.tile([128, 4, H, D + 1], BF16, tag="vaug", name="v_aug")
        nc.vector.memset(v_aug[:, :, :, D:], 1.0)
        W = C + (D + 1) * H  # 1032
        ag_in = dram.tile([TLOC, W], BF16, tag="agin", name="ag_in")
        ag_out = dram.tile([R * TLOC, W], BF16, tag="agout", name="ag_out",
                           addr_space="Shared")
        for blk in range(4, 8):  # k blocks: out [chan, t]
            ps = ps_ms.tile([128, TLOC], F32, tag="ms", name=f"psqk{blk}")
            for ks in range(4):
                nc.tensor.matmul(ps[:],
                                 wqkv_sb[:, ks, blk * 128:(blk + 1) * 128],
                                 xh1[:, ks], start=(ks == 0), stop=(ks == 3))
            nc.vector.tensor_scalar(kT_loc[:, blk % 4], ps[:],
                                    qkvb_sb[:, l, blk:blk + 1], None,
                                    mybir.AluOpType.add)
        for tt in range(4):  # v: out [t, chan]
            ps = ps_ms.tile([128, C], F32, tag="ms", name=f"psv{tt}")
            for ks in range(4):
                nc.tensor.matmul(ps[:], xh1[:, ks, tt * 128:(tt + 1) * 128],
                                 wqkv_sb[:, ks, 2 * C:3 * C],
                                 start=(ks == 0), stop=(ks == 3))
            nc.vector.tensor_scalar(
                v_aug[:, tt, :, 0:D], ps[:].rearrange("p (h d) -> p h d", h=H),
                qkvb_sb[:, l, 8:9], None, mybir.AluOpType.add)

        # ---- KV all-gather (overlaps with q computation) -------------------
        nc.sync.dma_start(
            ag_in[:, 0:C].rearrange("(s p) t -> p s t", p=128), kT_loc[:])
        nc.sync.dma_start(
            ag_in[:, C:W].rearrange("(s p) (h a) -> p s h a", p=128, h=H),
            v_aug[:])
        nc.gpsimd.collective_compute(
            "AllGather", mybir.AluOpType.bypass, replica_groups=rg,
            ins=[ag_in.opt()], outs=[ag_out.opt()])

        for blk in range(4):  # q blocks: out [chan, t]
            ps = ps_ms.tile([128, TLOC], F32, tag="ms", name=f"psq{blk}")
            for ks in range(4):
                nc.tensor.matmul(ps[:],
                                 wqkv_sb[:, ks, blk * 128:(blk + 1) * 128],
                                 xh1[:, ks], start=(ks == 0), stop=(ks == 3))
            nc.vector.tensor_scalar(qT[:, blk], ps[:],
                                    qkvb_sb[:, l, blk:blk + 1], None,
                                    mybir.AluOpType.add)

        yT = acts1.tile([128, 4, TLOC], BF16, tag="yT", name="yT")

        # diagonal score tiles from local K (computed while the AG runs)
        exd = acts1.tile([128, 2, H, 2, CH], BF16, tag="exd", name="exd")
        for slot in range(2):
            for h in range(H):
                po, sub = 64 * (h % 2), h // 2
                q_sl = qT[po:po + 64, sub, slot * CH:(slot + 1) * CH]
                for m in range(2):
                    ps_s = ps_sc.tile([128, CH], F32, tag="sc", name="ps_sd")
                    lhsT = kT_loc[po:po + 64, sub,
                                  slot * CH + m * 128:slot * CH + (m + 1) * 128]
                    nc.tensor.matmul(ps_s[:], lhsT, q_sl, start=True,
                                     stop=True)
                    nc.scalar.activation(
                        exd[:, slot, h, m], ps_s[:],
                        mybir.ActivationFunctionType.Exp,
                        bias=zero_sb[:], scale=0.125)
                    nc.vector.tensor_mul(exd[:, slot, h, m],
                                         exd[:, slot, h, m], tri_sb[:, m])

        for grp, ncg in ((0, KLO), (1, KHI)):
            k_g = kvp.tile([128, 4, ncg * CH], BF16, tag=f"kg{grp}",
                           name=f"k_g{grp}")
            v_g = kvp.tile([128, ncg, 2, H, D + 1], BF16, tag=f"vg{grp}",
                           name=f"v_g{grp}")
            kbase = 0 if grp == 0 else 2 * KLO
            vbase = 20 if grp == 0 else 20 + KLO
            for i in range(ncg):
                src_k = (ag_out[bass.ds(goff[kbase + 2 * i], 512)]
                         [:, bass.ds(goff[kbase + 2 * i + 1], CH)]
                         .rearrange("(s p) t -> p s t", p=128))
                nc.gpsimd.dma_start(k_g[:, :, i * CH:(i + 1) * CH], src_k)
                src_v = (ag_out[bass.ds(goff[vbase + i], CH), C:W]
                         .rearrange("(st p) (h a) -> p st h a", p=128, h=H))
                nc.gpsimd.dma_start(v_g[:, i], src_v)

            for h in range(H):
                po, sub = 64 * (h % 2), h // 2
                q_sl = qT[po:po + 64, sub, grp * CH:(grp + 1) * CH]
                ps_ya = ps_y.tile([128, D + 1], F32, tag="y", name="ps_ya")
                ps_yb = ps_y.tile([128, D + 1], F32, tag="y", name="ps_yb")
                n_st = 2 * ncg + 2
                for st in range(n_st):
                    if st < 2 * ncg:
                        ps_s = ps_sc.tile([128, CH], F32, tag="sc",
                                          name="ps_s")
                        lhsT = k_g[po:po + 64, sub, st * 128:(st + 1) * 128]
                        nc.tensor.matmul(ps_s[:], lhsT, q_sl, start=True,
                                         stop=True)
                        ex = expp.tile([128, CH], BF16, tag="ex", name="ex")
                        nc.scalar.activation(
                            ex[:], ps_s[:], mybir.ActivationFunctionType.Exp,
                            bias=bias_sb[:, grp, st:st + 1], scale=0.125)
                        v_sl = v_g[:, st // 2, st % 2, h, :]
                    else:
                        m = st - 2 * ncg
                        ex = exd[:, grp, h, m]
                        v_sl = v_aug[:, 2 * grp + m, h, :]
                    nc.tensor.matmul(ps_ya[:], ex[:, 0:128], v_sl,
                                     start=(st == 0), stop=(st == n_st - 1))
                    nc.tensor.matmul(ps_yb[:], ex[:, 128:256], v_sl,
                                     start=(st == 0), stop=(st == n_st - 1))
                for tb, ps_yt in enumerate((ps_ya, ps_yb)):
                    den = stat.tile([128, 1], F32, tag="den", name="den")
                    nc.vector.reciprocal(den[:], ps_yt[:, D:D + 1])
                    ysc = expp.tile([128, D], BF16, tag="ysc", name="ysc")
                    nc.vector.tensor_scalar(ysc[:], ps_yt[:, 0:D], den[:],
                                            None, mybir.AluOpType.mult)
                    pst = ps_ms.tile([64, 128], BF16, tag="ms", name="pst")
                    nc.tensor.transpose(pst[:], ysc[:], ident[:])
                    nc.vector.tensor_copy(
                        out=yT[po:po + 64, sub,
                               grp * CH + tb * 128:grp * CH + (tb + 1) * 128],
                        in_=pst[:])

        # ---- attn out proj + residual ------------------------------------
        for ob in range(4):
            ps = ps_ms.tile([128, TLOC], F32, tag="ms", name=f"psc{ob}")
            for ks in range(4):
                nc.tensor.matmul(ps[:], wc_sb[:, ks, ob * 128:(ob + 1) * 128],
                                 yT[:, ks], start=(ks == 0), stop=(ks == 3))
            nc.vector.tensor_add(xT[:, ob], xT[:, ob], ps[:])

        # ---- MLP ----------------------------------------------------------
        xh2 = acts.tile([128, 4, TLOC], BF16, tag="xh", name="xh2")
        layernorm(xT, xh2)
        hT = acts1.tile([128, 16, TLOC], BF16, tag="hT", name="hT")
        for fb in range(16):
            ps = ps_ms.tile([128, TLOC], F32, tag="ms", name=f"psf{fb}")
            for ks in range(4):
                nc.tensor.matmul(ps[:], wfc_sb[:, ks, fb * 128:(fb + 1) * 128],
                                 xh2[:, ks], start=(ks == 0), stop=(ks == 3))
            nc.scalar.activation(hT[:, fb], ps[:],
                                 mybir.ActivationFunctionType.Gelu,
                                 bias=fcb_sb[:, l, fb:fb + 1], scale=1.0)
        for ob in range(4):
            ps = ps_ms.tile([128, TLOC], F32, tag="ms", name=f"psp{ob}")
            for ks in range(16):
                nc.tensor.matmul(ps[:], wpr_sb[:, ks, ob * 128:(ob + 1) * 128],
                                 hT[:, ks], start=(ks == 0), stop=(ks == 15))
            nc.vector.tensor_add(xT[:, ob], xT[:, ob], ps[:])

    # ---- final LN + x all-gather -----------------------------------------
    xnf = acts.tile([128, 4, TLOC], BF16, tag="xh", name="xnf")
    layernorm(xT, xnf)
    ag2_in = dram.tile([C, TLOC], BF16, tag="ag2in", name="ag2_in")
    ag2_out = dram.tile([R * C, TLOC], BF16, tag="ag2out", name="ag2_out",
                        addr_space="Shared")
    nc.sync.dma_start(ag2_in.rearrange("(s p) t -> p s t", p=128), xnf[:])
    nc.gpsimd.collective_compute(
        "AllGather", mybir.AluOpType.bypass, replica_groups=rg,
        ins=[ag2_in.opt()], outs=[ag2_out.opt()])

    layer_ctx.close()

    # ---- lm_head ----------------------------------------------------------
    lm = ctx.enter_context(tc.tile_pool(name="lm", bufs=1))
    lmo = ctx.enter_context(tc.tile_pool(name="lmo", bufs=3))
    ps_lm = ctx.enter_context(tc.tile_pool(name="ps_lm", bufs=4, space="PSUM"))

    wte_sb = lm.tile([128, 4, VC], BF16, name="wte_sb")
    nc.sync.dma_start(wte_sb[:], wteT[:])
    xfT = lm.tile([128, 4, R, TLOC], BF16, name="xfT")
    ag2_v = ag2_out.rearrange("(r s p) t -> p s r t", p=128, s=4)
    for r in range(R):
        nc.sync.dma_start(xfT[:, :, r], ag2_v[:, :, r])

    NVC = (VC + 511) // 512  # 13
    for tt in range(R * TLOC // 128):  # 32 token tiles
        lsb = lmo.tile([128, VC], BF16, tag="lsb", name="lsb")
        for vb in range(NVC):
            v0 = vb * 512
            vw = min(512, VC - v0)
            ps = ps_lm.tile([128, 512], F32, tag="lm", name="pslm")
            for ks in range(4):
                nc.tensor.matmul(ps[:, 0:vw],
                                 xfT[:, ks, tt // 4,
                                     (tt % 4) * 128:(tt % 4 + 1) * 128],
                                 wte_sb[:, ks, v0:v0 + vw],
                                 start=(ks == 0), stop=(ks == 3))
            if vb % 2 == 0:
                nc.vector.tensor_copy(out=lsb[:, v0:v0 + vw],
                                      in_=ps[:, 0:vw])
            else:
                nc.scalar.copy(out=lsb[:, v0:v0 + vw], in_=ps[:, 0:vw])
        nc.sync.dma_start(logits[tt * 128:(tt + 1) * 128, :], lsb[:])

    ctx.close()


# ---------------------------------------------------------------------------
# Host side
# ---------------------------------------------------------------------------

_CACHE = {}


def _prep_inputs(idx, wte, wpe, ln1_w, ln1_b, c_attn_w, c_proj_w, ln2_w,
                 ln2_b, fc_w, proj_w, lnf_w, lnf_b):
    idx = np.asarray(idx)
    f32 = lambda a: np.asarray(a, np.float32)
    wte, wpe = f32(wte), f32(wpe)
    ln1_w, ln1_b, ln2_w, ln2_b = f32(ln1_w), f32(ln1_b), f32(ln2_w), f32(ln2_b)
    lnf_w, lnf_b = f32(lnf_w), f32(lnf_b)
    c_attn_w, c_proj_w = f32(c_attn_w), f32(c_proj_w)
    fc_w, proj_w = f32(fc_w), f32(proj_w)

    # biases that cannot be folded per-partition must be zero (they are in
    # the reference: all LayerNorm biases are zeros)
    assert np.all(lnf_b == 0.0), "non-zero lnf_b not supported"

    # local token order: [LO chunk (position <= 3), HI chunk (position >= 4)]
    tok_of_core, lohi = [], []
    for r in range(R):
        ca, cb = r, 7 - r  # batch0 position, batch1 position
        t_b0 = np.arange(ca * CH, (ca + 1) * CH)
        t_b1 = T + np.arange(cb * CH, (cb + 1) * CH)
        if ca <= 3:
            tok_of_core.append(np.concatenate([t_b0, t_b1]))
            lohi.append((0, ca, 1, cb))  # (blo, plo, bhi, phi)
        else:
            tok_of_core.append(np.concatenate([t_b1, t_b0]))
            lohi.append((1, cb, 0, ca))
    gather_order = np.concatenate(tok_of_core)

    def _owner_col(b, pos):
        """ag_out row-block owner core + local column of chunk (b, pos)."""
        o = pos if b == 0 else 7 - pos
        olo = (0, o) if o <= 3 else (1, 7 - o)
        return o, (0 if (b, pos) == olo else CH)

    x0 = wte[idx.reshape(-1)] + np.tile(wpe[:T], (B, 1))  # [4096, C]

    wte_pad = np.zeros((VC * R, C), np.float32)
    wte_pad[:V] = wte * lnf_w[None, :]  # fold lnf scale into tied head

    # shared (core-independent) weight prep
    wqkvT = np.zeros((L, 128, 4, 3 * C), bfloat16)
    wcT = np.zeros((L, 128, 4, C), bfloat16)
    wfcT = np.zeros((L, 128, 4, FF), bfloat16)
    wprT = np.zeros((L, 128, 16, C), bfloat16)
    qkv_b = np.zeros((L, 128, 9), np.float32)
    fc_b = np.zeros((L, 128, 16), np.float32)
    for l in range(L):
        wq = c_attn_w[l] * ln1_w[l][None, :]
        bq = c_attn_w[l] @ ln1_b[l]  # [3C]
        wqkvT[l] = _to_cpart(wq.T).astype(bfloat16)
        for blk in range(8):
            qkv_b[l, :, blk] = bq[blk * 128:(blk + 1) * 128]
        assert np.allclose(bq[2 * C:], bq[2 * C]), "v bias must be uniform"
        qkv_b[l, :, 8] = bq[2 * C]
        wcT[l] = _to_cpart(c_proj_w[l].T).astype(bfloat16)
        wf = fc_w[l] * ln2_w[l][None, :]
        bf = fc_w[l] @ ln2_b[l]  # [FF]
        wfcT[l] = _to_cpart(wf.T).astype(bfloat16)
        fc_b[l] = bf.reshape(16, 128).T
        wprT[l] = _to_cpart(proj_w[l].T).astype(bfloat16)

    tri = np.zeros((128, 2, CH), bfloat16)
    for m in range(2):
        sp = m * 128 + np.arange(128)
        tri[:, m, :] = (sp[:, None] <= np.arange(CH)[None, :]).astype(
            np.float32)

    in_maps = []
    for r in range(R):
        toks = tok_of_core[r]
        x0T = _to_cpart(np.ascontiguousarray(x0[toks].T))

        blo, plo, bhi, phi = lohi[r]
        ab = np.zeros((128, 2, 14), np.float32)
        for i in range(6):
            ab[:, 0, i] = 0.0 if i // 2 < plo else NEG
        for i in range(14):
            ab[:, 1, i] = 0.0 if i // 2 < phi else NEG

        off = np.zeros((1, 32), np.int32)
        for i in range(3):
            pos = i if i < plo else 0
            o, col = _owner_col(blo, pos)
            off[0, 2 * i], off[0, 2 * i + 1] = o * 512, col
            off[0, 20 + i] = o * 512 + col
        for i in range(7):
            pos = i if i < phi else 0
            o, col = _owner_col(bhi, pos)
            off[0, 6 + 2 * i], off[0, 6 + 2 * i + 1] = o * 512, col
            off[0, 23 + i] = o * 512 + col

        wteTr = _to_cpart(
            np.ascontiguousarray(wte_pad[r * VC:(r + 1) * VC].T)
        ).astype(bfloat16)

        in_maps.append({
            "x0T": x0T, "wqkvT": wqkvT, "wcT": wcT, "wfcT": wfcT,
            "wprT": wprT, "qkv_b": qkv_b, "fc_b": fc_b,
            "attn_bias": ab, "trimask": tri, "wteT": wteTr,
            "gath_off": off,
        })
    return in_maps, gather_order


def kernel(**inputs):
    if "nc" not in _CACHE:
        _CACHE["nc"] = build_program()
    nc = _CACHE["nc"]

    in_maps, gather_order = _prep_inputs(**inputs)
    res = run_bass_kernel_spmd(nc, in_maps, core_ids=list(range(R)))
    _CACHE["res"] = res  # exec_time_ns etc when BASS_TRACE=1
    outs = [np.asarray(res.results[r]["logits"]) for r in range(R)]
    full = np.concatenate(outs, axis=1)  # [4096, R*VC] gathered row order
    inv = np.empty(B * T, np.int64)
    inv[gather_order] = np.arange(B * T)
    full = full[inv][:, :V].astype(np.float32)
    return full.reshape(B, T, V)
